# revision 1
# baseline (speedup 1.0000x reference)
"""CapsuleLayer dynamic-routing kernel for 8 Trainium2 NeuronCores.

Problem: x [64,2048,16], route_weights [32,2048,16,32] ->
  3-iteration routing -> out [32,64,1,1,32] (fp32).

Sharding: capsules (C=32) split 4-per-core across 8 cores; x replicated.
Per core everything is dense matmuls + DVE elementwise:

  priors[c,b,r,o] = sum_j x[b,r,j] W[c,r,j,o]
  s1 = mean_r priors              -> one big PE contraction over (j,r)
  V_i[c,b,(r,j)] = sum_o W[c,r,j,o] out_i[c,b,o]   (PE, K=o=32, 4-way packed)
  d_i[c,b,r] = sum_j x[b,(r,j)] V_i[c,b,(r,j)]     (DVE mult + grouped reduce)
  logits += d_i ; e = exp(logits - max)            (ACT)
  xe[c][(j,r),b] = xt2[(j,r),b] * eT[c][r,b]       (DVE; j-blocked layout)
  s_{i+1}[c,b,o] = (sum_{(j,r)} xe W) / Z          (PE, K=(j,r))
  out_i = squash(s_i)
"""
import os
import numpy as np

C, B, R, CIN, OUT = 32, 64, 2048, 16, 32
NCORES = 8
CLOC = C // NCORES          # 4 capsules per core
RJ = R * CIN                # 32768
NK = RJ // 128              # 256 chunks of 128 along (j,r) / (r,j)

_CACHE = {}


def _build_program():
    from contextlib import ExitStack
    import concourse.bass as bass
    import concourse.bacc as bacc
    import concourse.tile as tile
    from concourse import mybir

    f32 = mybir.dt.float32
    AL = mybir.AluOpType
    AF = mybir.ActivationFunctionType
    AX = mybir.AxisListType

    nc = bacc.Bacc(None, target_bir_lowering=False,
                   detect_race_conditions=not bool(int(os.environ.get("CAPS_NO_RACE", "0"))))
    n_loops = int(os.environ.get("CAPS_LOOPS", "1"))
    v32r = bool(int(os.environ.get("CAPS_V32R", "0")))
    vsp3 = bool(int(os.environ.get("CAPS_VSPLIT3", "0")))
    f32r = mybir.dt.float32r
    bf16 = mybir.dt.bfloat16

    # ---- DRAM I/O ----
    w2cat = nc.dram_tensor("w2cat", [RJ, 128], f32, kind="ExternalInput")     # [(j,r),(c,o)]
    xt2 = nc.dram_tensor("xt2", [RJ, B], f32, kind="ExternalInput")           # [(j,r),b]
    wt = nc.dram_tensor("wt", [CLOC, OUT, RJ], f32, kind="ExternalInput")     # [c,o,(r,j)]
    if vsp3:
        wth = nc.dram_tensor("wth", [CLOC, OUT, RJ], bf16, kind="ExternalInput")
        wtl = nc.dram_tensor("wtl", [CLOC, OUT, RJ], bf16, kind="ExternalInput")
    x2d = nc.dram_tensor("x2d", [128, RJ], f32, kind="ExternalInput")         # [(2,b),(r,j)]
    ident = nc.dram_tensor("ident", [128, 128], f32, kind="ExternalInput")
    out3 = nc.dram_tensor("out3", [B, 128], f32, kind="ExternalOutput")       # [b,(c,o)]

    with tile.TileContext(nc) as tc, ExitStack() as ctx:
        const = ctx.enter_context(tc.tile_pool(name="const", bufs=1))
        small = ctx.enter_context(tc.tile_pool(name="small", bufs=3))
        wcat_p = ctx.enter_context(tc.tile_pool(name="wcat", bufs=4))
        wt_p = ctx.enter_context(tc.tile_pool(name="wtp", bufs=3))
        big = ctx.enter_context(tc.tile_pool(name="big", bufs=2))
        xe_p = ctx.enter_context(tc.tile_pool(name="xep", bufs=2))
        psacc_p = ctx.enter_context(tc.tile_pool(name="psacc", bufs=4, space="PSUM"))
        psV_p = ctx.enter_context(tc.tile_pool(name="psV", bufs=4, space="PSUM"))
        psT_p = psV_p

        idn = const.tile([128, 128], f32, tag="ident", name="idn")
        nc.sync.dma_start(out=idn, in_=ident[:])

        # resident xt2: [128, (k=256, b=64)]
        xt2_sb = const.tile([128, NK, B], f32, tag="xt2sb", name="xt2_sb")
        nc.sync.dma_start(out=xt2_sb, in_=xt2[:].rearrange("(k p) b -> p k b", p=128))

        # logits per capsule-pair [(2c,b)=128, r=2048]
        lP = [const.tile([128, R], f32, tag=f"l{p}", name=f"lP{p}") for p in range(2)]
        # transposed-unnormalized-probs  [128=r%128, (c=4, rb=16, b=64)]
        p2T = const.tile([128, CLOC, R // 128, B], f32, tag="p2T", name="p2T")

        def squash(u_bT, scale_pow):
            """u_bT [64,(4c,32o)]: s = u*scale_pow; out = s*sqrt(n2)/(n2+1)."""
            sq = small.tile([B, 128], f32, tag="sq", name="sq")
            nc.vector.scalar_tensor_tensor(
                out=sq, in0=u_bT, scalar=float(scale_pow * scale_pow),
                in1=u_bT, op0=AL.mult, op1=AL.mult)
            n2 = small.tile([B, CLOC], f32, tag="n2", name="n2")
            nc.vector.tensor_reduce(
                out=n2, in_=sq[:].rearrange("b (c o) -> b c o", c=CLOC),
                axis=AX.X, op=AL.add)
            rt = small.tile([B, CLOC], f32, tag="rt", name="rt")
            nc.scalar.activation(out=rt, in_=n2, func=AF.Sqrt)
            dn = small.tile([B, CLOC], f32, tag="dn", name="dn")
            nc.vector.tensor_scalar_add(out=dn, in0=n2, scalar1=1.0)
            rc = small.tile([B, CLOC], f32, tag="rc", name="rc")
            nc.vector.reciprocal(out=rc, in_=dn)
            f = small.tile([B, CLOC], f32, tag="f", name="f")
            nc.vector.tensor_mul(out=f, in0=rt, in1=rc)
            f2 = small.tile([B, CLOC], f32, tag="f2", name="f2")
            nc.vector.tensor_scalar_mul(out=f2, in0=f, scalar1=float(scale_pow))
            o_i = small.tile([B, 128], f32, tag="oi", name="oi")
            f2b = bass.AP(tensor=f2[:].tensor, offset=f2[:].offset,
                          ap=[f2[:].ap[0], f2[:].ap[1], [0, OUT]])
            nc.vector.tensor_tensor(
                out=o_i[:].rearrange("b (c o) -> b c o", c=CLOC),
                in0=u_bT[:].rearrange("b (c o) -> b c o", c=CLOC),
                in1=f2b, op=AL.mult)
            psOT = psT_p.tile([128, B], f32, tag="psVT", name="psOT")
            nc.tensor.transpose(psOT, o_i, idn[0:B, 0:B])
            oT = small.tile([128, B], f32, tag="oT", name="oT")
            nc.scalar.copy(out=oT, in_=psOT)
            if v32r:
                oTr = small.tile([128, B], f32r, tag="oTr", name="oTr")
                nc.gpsimd.dma_start(out=oTr, in_=oT)
                return o_i, oTr
            if vsp3:
                oTh = small.tile([128, B], bf16, tag="oTh", name="oTh")
                nc.vector.tensor_copy(out=oTh, in_=oT)
                dfh = small.tile([128, B], f32, tag="dfh", name="dfh")
                nc.vector.tensor_sub(out=dfh, in0=oT, in1=oTh)
                oTl = small.tile([128, B], bf16, tag="oTl", name="oTl")
                nc.vector.tensor_copy(out=oTl, in_=dfh)
                return o_i, (oTh, oTl)
            return o_i, oT

        for _loop in range(n_loops):
            # ---------- Phase A: s1 = (1/R) sum_(j,r) x W ----------
            psA = psacc_p.tile([128, B], f32, tag="acc", name="psA")
            for k in range(NK):
                wck = wcat_p.tile([128, 128], f32, tag="wck", name="wck")
                nc.sync.dma_start(out=wck, in_=w2cat[128 * k:128 * (k + 1), :])
                nc.tensor.matmul(psA, wck, xt2_sb[:, k, :],
                                 start=(k == 0), stop=(k == NK - 1))
            sA = small.tile([128, B], f32, tag="sA", name="sA")
            nc.scalar.copy(out=sA, in_=psA)
            psAT = psT_p.tile([B, 128], f32, tag="psVT", name="psAT")
            nc.tensor.transpose(psAT, sA, idn)
            uT = small.tile([B, 128], f32, tag="uT", name="uT")
            nc.scalar.copy(out=uT, in_=psAT)
            out_i, outT = squash(uT, 1.0 / R)

            # ---------- Two routing boundaries ----------
            for it in (1, 2):
                # --- V + delta ---
                for g in range(16):
                    x2k = big.tile([128, 2048], f32, tag="x2k", name="x2k")
                    nc.sync.dma_start(out=x2k, in_=x2d[:, 2048 * g:2048 * (g + 1)])
                    vs = [big.tile([128, 2048], f32, tag="vs", name=f"vs{p}")
                          for p in range(2)]
                    for t in range(4):
                        k = 4 * g + t
                        if vsp3:
                            wtkh = wt_p.tile([128, 512], bf16, tag="wtkh", name="wtkh")
                            nc.sync.dma_start(
                                out=wtkh,
                                in_=wth[:, :, 512 * k:512 * (k + 1)].rearrange(
                                    "c o n -> (c o) n"))
                            wtkl = wt_p.tile([128, 512], bf16, tag="wtkl", name="wtkl")
                            nc.sync.dma_start(
                                out=wtkl,
                                in_=wtl[:, :, 512 * k:512 * (k + 1)].rearrange(
                                    "c o n -> (c o) n"))
                        else:
                            wtk = wt_p.tile([128, 512], f32r if v32r else f32,
                                            tag="wtk", name="wtk")
                            eng = nc.gpsimd if v32r else nc.sync
                            eng.dma_start(
                                out=wtk,
                                in_=wt[:, :, 512 * k:512 * (k + 1)].rearrange(
                                    "c o n -> (c o) n"))
                        if vsp3:
                            oTh, oTl = outT
                            psV4 = [psV_p.tile([B, 512], f32, tag="psVT",
                                               name=f"psV4_{c}") for c in range(CLOC)]
                            for c4 in range(CLOC):
                                sl = slice(32 * c4, 32 * (c4 + 1))
                                tp = (32 * c4, 0)
                                nc.tensor.matmul(psV4[c4], oTh[sl, :], wtkh[sl, :],
                                                 start=True, stop=False,
                                                 tile_position=tp)
                                nc.tensor.matmul(psV4[c4], oTl[sl, :], wtkh[sl, :],
                                                 start=False, stop=False,
                                                 tile_position=tp)
                                nc.tensor.matmul(psV4[c4], oTh[sl, :], wtkl[sl, :],
                                                 start=False, stop=True,
                                                 tile_position=tp)
                            for c4 in range(CLOC):
                                pr, ce = divmod(c4, 2)
                                nc.scalar.copy(
                                    out=vs[pr][64 * ce:64 * (ce + 1),
                                               512 * t:512 * (t + 1)],
                                    in_=psV4[c4])
                        elif v32r:
                            psV4 = [psV_p.tile([B, 512], f32, tag="psVT",
                                               name=f"psV4_{c}") for c in range(CLOC)]
                            for c4 in range(CLOC):
                                nc.tensor.matmul(
                                    psV4[c4],
                                    outT[32 * c4:32 * (c4 + 1), :],
                                    wtk[32 * c4:32 * (c4 + 1), :],
                                    start=True, stop=True,
                                    tile_position=(32 * c4, 0))
                            for c4 in range(CLOC):
                                pr, ce = divmod(c4, 2)
                                nc.scalar.copy(
                                    out=vs[pr][64 * ce:64 * (ce + 1),
                                               512 * t:512 * (t + 1)],
                                    in_=psV4[c4])
                        else:
                            psV = [psV_p.tile([128, 512], f32, tag="psVT",
                                              name=f"psV{p}") for p in range(2)]
                            for c4 in range(CLOC):
                                pr, ce = divmod(c4, 2)
                                nc.tensor.matmul(
                                    psV[pr][64 * ce:64 * (ce + 1), :],
                                    outT[32 * c4:32 * (c4 + 1), :],
                                    wtk[32 * c4:32 * (c4 + 1), :],
                                    start=True, stop=True,
                                    tile_position=(32 * c4, 64 * ce))
                            for pr in range(2):
                                nc.scalar.copy(out=vs[pr][:, 512 * t:512 * (t + 1)],
                                               in_=psV[pr])
                    for pr in range(2):
                        nc.vector.tensor_mul(out=vs[pr], in0=vs[pr], in1=x2k)
                        if it == 1:
                            nc.vector.tensor_reduce(
                                out=lP[pr][:, 128 * g:128 * (g + 1)],
                                in_=vs[pr][:].rearrange("p (r j) -> p r j", j=CIN),
                                axis=AX.X, op=AL.add)
                        else:
                            dtmp = small.tile([128, 128], f32, tag="dtmp", name="dtmp")
                            nc.vector.tensor_reduce(
                                out=dtmp,
                                in_=vs[pr][:].rearrange("p (r j) -> p r j", j=CIN),
                                axis=AX.X, op=AL.add)
                            nc.vector.tensor_add(
                                out=lP[pr][:, 128 * g:128 * (g + 1)],
                                in0=lP[pr][:, 128 * g:128 * (g + 1)], in1=dtmp)

                # --- softmax pieces (unnormalized e + Z) ---
                zq = small.tile([B, CLOC], f32, tag="zq", name="zq")
                for pr in range(2):
                    m = small.tile([128, 1], f32, tag="m", name="m")
                    nc.vector.tensor_reduce(out=m, in_=lP[pr], axis=AX.X, op=AL.max)
                    mneg = small.tile([128, 1], f32, tag="mneg", name="mneg")
                    nc.vector.tensor_scalar_mul(out=mneg, in0=m, scalar1=-1.0)
                    eP = big.tile([128, R], f32, tag="e", name="eP")
                    Z = small.tile([128, 1], f32, tag="Z", name="Z")
                    nc.scalar.activation(out=eP, in_=lP[pr], func=AF.Exp,
                                         bias=mneg[:, 0:1], scale=1.0, accum_out=Z)
                    for ce in range(2):
                        nc.sync.dma_start(out=zq[:, 2 * pr + ce:2 * pr + ce + 1],
                                          in_=Z[64 * ce:64 * (ce + 1), 0:1])
                        for rb in range(R // 128):
                            psT2 = psT_p.tile([128, B], f32, tag="psVT", name="psT2")
                            nc.tensor.transpose(
                                psT2,
                                eP[64 * ce:64 * (ce + 1), 128 * rb:128 * (rb + 1)],
                                idn[64 * ce:64 * ce + 64, 64 * ce:64 * ce + 64])
                            nc.scalar.copy(out=p2T[:, 2 * pr + ce, rb, :], in_=psT2)
                rzq = small.tile([B, CLOC], f32, tag="rzq", name="rzq")
                nc.vector.reciprocal(out=rzq, in_=zq)

                # --- xe + s matmuls ---
                psS = [psacc_p.tile([B, 32], f32, tag="acc", name=f"psS{c}")
                       for c in range(CLOC)]
                for j in range(CIN):
                    xes = []
                    for c4 in range(CLOC):
                        xe = xe_p.tile([128, R // 128, B], f32, tag=f"xe{c4}",
                                       name=f"xe{c4}")
                        nc.vector.tensor_mul(
                            out=xe,
                            in0=xt2_sb[:, 16 * j:16 * (j + 1), :],
                            in1=p2T[:, c4, :, :])
                        xes.append(xe)
                    for t in range(R // 128):
                        k = 16 * j + t
                        wck = wcat_p.tile([128, 128], f32, tag="wck", name="wck")
                        nc.sync.dma_start(out=wck, in_=w2cat[128 * k:128 * (k + 1), :])
                        for c4 in range(CLOC):
                            nc.tensor.matmul(
                                psS[c4],
                                xes[c4][:, t, :],
                                wck[:, 32 * c4:32 * (c4 + 1)],
                                start=(k == 0), stop=(k == NK - 1))
                sS = small.tile([B, 128], f32, tag="sS", name="sS")
                for c4 in range(CLOC):
                    nc.scalar.activation(out=sS[:, 32 * c4:32 * (c4 + 1)],
                                         in_=psS[c4],
                                         func=AF.Copy, bias=0.0,
                                         scale=rzq[:, c4:c4 + 1])
                out_i, outT = squash(sS, 1.0)

            nc.sync.dma_start(out=out3[:], in_=out_i)

    nc.finalize()
    return nc


def _get_program():
    if "nc" not in _CACHE:
        _CACHE["nc"] = _build_program()
    return _CACHE["nc"]


def make_in_maps(x, route_weights):
    import ml_dtypes
    vsp3 = bool(int(os.environ.get("CAPS_VSPLIT3", "0")))
    x = np.ascontiguousarray(x, dtype=np.float32)
    W = np.ascontiguousarray(route_weights, dtype=np.float32)
    xt2 = np.ascontiguousarray(x.transpose(2, 1, 0).reshape(RJ, B))       # [(j,r),b]
    xnat = x.reshape(B, RJ)                                               # [b,(r,j)]
    x2d = np.ascontiguousarray(np.concatenate([xnat, xnat], axis=0))      # [128,(r,j)]
    ident = np.eye(128, dtype=np.float32)
    in_maps = []
    for core in range(NCORES):
        wc = W[CLOC * core:CLOC * (core + 1)]                             # [4,R,J,O]
        wtc = np.ascontiguousarray(
            wc.transpose(0, 3, 1, 2).reshape(CLOC, OUT, RJ))              # [c,o,(r,j)]
        m = {"w2cat": np.ascontiguousarray(
                wc.transpose(2, 1, 0, 3).reshape(RJ, CLOC * OUT)),        # [(j,r),(c,o)]
             "xt2": xt2, "wt": wtc, "x2d": x2d, "ident": ident}
        if vsp3:
            wth = wtc.astype(ml_dtypes.bfloat16)
            m["wth"] = wth
            m["wtl"] = (wtc - wth.astype(np.float32)).astype(ml_dtypes.bfloat16)
        in_maps.append(m)
    return in_maps


def kernel(x, route_weights):
    from concourse.bass_utils import run_bass_kernel_spmd

    in_maps = make_in_maps(x, route_weights)
    nc = _get_program()
    res = run_bass_kernel_spmd(nc, in_maps, core_ids=list(range(NCORES)))
    if os.environ.get("CAPS_RESULT_STASH"):
        _CACHE["last_result"] = res

    out = np.empty((C, B, 1, 1, OUT), dtype=np.float32)
    for core in range(NCORES):
        o = res.results[core]["out3"].reshape(B, CLOC, OUT).transpose(1, 0, 2)
        out[CLOC * core:CLOC * (core + 1), :, 0, 0, :] = o
    return out



# revision 5
# speedup vs baseline: 3.3022x; 3.3022x over previous
"""CapsuleLayer dynamic-routing kernel for 8 Trainium2 NeuronCores (v2).

Problem: x [64,2048,16], route_weights [32,2048,16,32] ->
  3-iteration routing -> out [32,64,1,1,32] (fp32).

Sharding: capsules (C=32) split 4-per-core across 8 cores; x replicated.

v2 design (fp16 datapath, fp32 accumulation):
  - w2cat ([(j,r),(c,o)]) and xt2 ([(j,r),b]) resident in SBUF as fp16;
    wt ([(c,o),(r,j)]) and x2d ([(2,b),(r,j)]) streamed per boundary.
  - Phase A: psA[b,co] += xt2_k^T @ w2cat_k  (stationary xt2, no transpose).
  - V-matmul: quadrant-packed (tile_position) K=32 matmuls, N=512 fp16.
  - delta: DVE fp16 multiply (2x mode) + grouped reduce over j.
  - softmax: DVE max, ACT exp with fp16 out + accumulated Z.
  - probs transpose: full 128x128 PE transposes (2 per pr per rb-block).
  - s-matmul: col-packed (tile_position=(0,32c)) 4-capsule matmuls
    accumulating into one psS[(c,o),b] PSUM tile; 1/Z folded into squash.
"""
import os
import numpy as np

C, B, R, CIN, OUT = 32, 64, 2048, 16, 32
NCORES = 8
CLOC = C // NCORES          # 4 capsules per core
RJ = R * CIN                # 32768
NK = RJ // 128              # 256 chunks of 128 along (j,r) / (r,j)
NG = 16                     # rj chunks of 2048 along (r,j)

_CACHE = {}


def _build_program():
    from contextlib import ExitStack
    import concourse.bass as bass
    import concourse.bacc as bacc
    import concourse.tile as tile
    from concourse import mybir

    f32 = mybir.dt.float32
    f16 = mybir.dt.float16
    AL = mybir.AluOpType
    AF = mybir.ActivationFunctionType
    AX = mybir.AxisListType

    nc = bacc.Bacc(None, target_bir_lowering=False,
                   detect_race_conditions=not bool(int(os.environ.get("CAPS_NO_RACE", "0"))))
    n_loops = int(os.environ.get("CAPS_LOOPS", "1"))

    # ---- DRAM I/O ----
    # resident (host pre-rearranged):
    w2r = nc.dram_tensor("w2r", [128, NK, 128], f16, kind="ExternalInput")   # [rj%128, k, (c,o)]
    xt2r = nc.dram_tensor("xt2r", [128, NK, B], f16, kind="ExternalInput")   # [rj%128, k, b]
    # streamed:
    wt = nc.dram_tensor("wt", [128, RJ], f16, kind="ExternalInput")          # [(c,o), (r,j)]
    x2d = nc.dram_tensor("x2d", [128, RJ], f16, kind="ExternalInput")        # [(2,b), (r,j)]
    ident = nc.dram_tensor("ident", [128, 128], f32, kind="ExternalInput")
    out3 = nc.dram_tensor("out3", [B, 128], f32, kind="ExternalOutput")      # [b, (c,o)]

    with tile.TileContext(nc) as tc, ExitStack() as ctx:
        const = ctx.enter_context(tc.tile_pool(name="const", bufs=1))
        small = ctx.enter_context(tc.tile_pool(name="small", bufs=3))
        wtg_p = ctx.enter_context(tc.tile_pool(name="wtg", bufs=3))
        x2k_p = ctx.enter_context(tc.tile_pool(name="x2k", bufs=3))
        vs_p = ctx.enter_context(tc.tile_pool(name="vs", bufs=3))
        xe_p = ctx.enter_context(tc.tile_pool(name="xe", bufs=2))
        eP_p = ctx.enter_context(tc.tile_pool(name="eP", bufs=2))
        psV_p = ctx.enter_context(tc.tile_pool(name="psV", bufs=3, space="PSUM"))
        psacc_p = ctx.enter_context(tc.tile_pool(name="psacc", bufs=1, space="PSUM"))
        psT_p = ctx.enter_context(tc.tile_pool(name="psT", bufs=2, space="PSUM"))

        idn = const.tile([128, 128], f32, tag="ident", name="idn")
        nc.sync.dma_start(out=idn, in_=ident[:])
        idn16 = const.tile([128, 128], f16, tag="ident16", name="idn16")
        nc.vector.tensor_copy(out=idn16, in_=idn)

        # resident fp16 tensors
        w2_sb = const.tile([128, NK, 128], f16, tag="w2sb", name="w2_sb")
        nc.sync.dma_start(out=w2_sb, in_=w2r[:])
        xt2_sb = const.tile([128, NK, B], f16, tag="xt2sb", name="xt2_sb")
        nc.sync.dma_start(out=xt2_sb, in_=xt2r[:])

        # logits per capsule-pair [(ce,b)=128, r=2048] fp32
        lP = [const.tile([128, R], f32, tag=f"l{p}", name=f"lP{p}") for p in range(2)]
        # transposed unnormalized probs [r%128, c4, rb, b] fp16
        p2T = const.tile([128, CLOC, R // 128, B], f16, tag="p2T", name="p2T")

        def squash(u_bT, rz=None, scale_pow=1.0):
            """u_bT [64,(4c,32o)] f32. If rz given ([64,4] f32 per-(b,c)
            scale), squash(u*rz); else squash(u*scale_pow)."""
            sq = small.tile([B, 128], f32, tag="sq", name="sq")
            n2 = small.tile([B, CLOC], f32, tag="n2", name="n2")
            if rz is None:
                nc.vector.scalar_tensor_tensor(
                    out=sq, in0=u_bT, scalar=float(scale_pow * scale_pow),
                    in1=u_bT, op0=AL.mult, op1=AL.mult)
                nc.vector.tensor_reduce(
                    out=n2, in_=sq[:].rearrange("b (c o) -> b c o", c=CLOC),
                    axis=AX.X, op=AL.add)
            else:
                nc.vector.tensor_mul(out=sq, in0=u_bT, in1=u_bT)
                q2 = small.tile([B, CLOC], f32, tag="q2", name="q2")
                nc.vector.tensor_reduce(
                    out=q2, in_=sq[:].rearrange("b (c o) -> b c o", c=CLOC),
                    axis=AX.X, op=AL.add)
                rz2 = small.tile([B, CLOC], f32, tag="rz2", name="rz2")
                nc.vector.tensor_mul(out=rz2, in0=rz, in1=rz)
                nc.vector.tensor_mul(out=n2, in0=q2, in1=rz2)
            rt = small.tile([B, CLOC], f32, tag="rt", name="rt")
            nc.scalar.activation(out=rt, in_=n2, func=AF.Sqrt)
            dn = small.tile([B, CLOC], f32, tag="dn", name="dn")
            nc.vector.tensor_scalar_add(out=dn, in0=n2, scalar1=1.0)
            rc = small.tile([B, CLOC], f32, tag="rc", name="rc")
            nc.vector.reciprocal(out=rc, in_=dn)
            f = small.tile([B, CLOC], f32, tag="f", name="f")
            nc.vector.tensor_mul(out=f, in0=rt, in1=rc)
            f2 = small.tile([B, CLOC], f32, tag="f2", name="f2")
            if rz is None:
                nc.vector.tensor_scalar_mul(out=f2, in0=f, scalar1=float(scale_pow))
            else:
                nc.vector.tensor_mul(out=f2, in0=f, in1=rz)
            o_i = small.tile([B, 128], f32, tag="oi", name="oi")
            f2b = bass.AP(tensor=f2[:].tensor, offset=f2[:].offset,
                          ap=[f2[:].ap[0], f2[:].ap[1], [0, OUT]])
            nc.vector.tensor_tensor(
                out=o_i[:].rearrange("b (c o) -> b c o", c=CLOC),
                in0=u_bT[:].rearrange("b (c o) -> b c o", c=CLOC),
                in1=f2b, op=AL.mult)
            psOT = psT_p.tile([128, B], f32, tag="psT", name="psOT")
            nc.tensor.transpose(psOT, o_i, idn[0:B, 0:B])
            oT = small.tile([128, B], f16, tag="oT", name="oT")
            nc.scalar.copy(out=oT, in_=psOT)
            return o_i, oT

        for _loop in range(n_loops):
            # ---------- Phase A: s1 = (1/R) sum_(j,r) x W ----------
            psA = psacc_p.tile([B, 128], f32, tag="psA", name="psA")
            for k in range(NK):
                nc.tensor.matmul(psA, xt2_sb[:, k, :], w2_sb[:, k, :],
                                 start=(k == 0), stop=(k == NK - 1))
            uA = small.tile([B, 128], f32, tag="uA", name="uA")
            nc.scalar.copy(out=uA, in_=psA)
            out_i, outT = squash(uA, scale_pow=1.0 / R)

            # ---------- Two routing boundaries ----------
            for it in (1, 2):
                # --- V + delta ---
                for g in range(NG):
                    wtg = wtg_p.tile([128, 2048], f16, tag="wtg", name="wtg")
                    nc.sync.dma_start(out=wtg, in_=wt[:, 2048 * g:2048 * (g + 1)])
                    x2k = x2k_p.tile([128, 2048], f16, tag="x2k", name="x2k")
                    nc.sync.dma_start(out=x2k, in_=x2d[:, 2048 * g:2048 * (g + 1)])
                    vs = [vs_p.tile([128, 2048], f16, tag="vs", name=f"vs{p}")
                          for p in range(2)]
                    for t in range(4):
                        psV = [psV_p.tile([128, 512], f32, tag="psV",
                                          name=f"psV{p}") for p in range(2)]
                        for c4 in range(CLOC):
                            pr, ce = divmod(c4, 2)
                            nc.tensor.matmul(
                                psV[pr][64 * ce:64 * (ce + 1), :],
                                outT[32 * c4:32 * (c4 + 1), :],
                                wtg[32 * c4:32 * (c4 + 1),
                                    512 * t:512 * (t + 1)],
                                start=True, stop=True,
                                tile_position=(32 * c4, 64 * ce))
                        for pr in range(2):
                            nc.scalar.copy(out=vs[pr][:, 512 * t:512 * (t + 1)],
                                           in_=psV[pr])
                    for pr in range(2):
                        nc.vector.tensor_mul(out=vs[pr], in0=vs[pr], in1=x2k)
                        if it == 1:
                            nc.vector.tensor_reduce(
                                out=lP[pr][:, 128 * g:128 * (g + 1)],
                                in_=vs[pr][:].rearrange("p (r j) -> p r j", j=CIN),
                                axis=AX.X, op=AL.add)
                        else:
                            dtmp = small.tile([128, 128], f32, tag="dtmp", name="dtmp")
                            nc.vector.tensor_reduce(
                                out=dtmp,
                                in_=vs[pr][:].rearrange("p (r j) -> p r j", j=CIN),
                                axis=AX.X, op=AL.add)
                            nc.vector.tensor_add(
                                out=lP[pr][:, 128 * g:128 * (g + 1)],
                                in0=lP[pr][:, 128 * g:128 * (g + 1)], in1=dtmp)

                # --- softmax pieces (unnormalized e + Z) ---
                zq = small.tile([B, CLOC], f32, tag="zq", name="zq")
                for pr in range(2):
                    m = small.tile([128, 1], f32, tag="m", name="m")
                    nc.vector.tensor_reduce(out=m, in_=lP[pr], axis=AX.X, op=AL.max)
                    mneg = small.tile([128, 1], f32, tag="mneg", name="mneg")
                    nc.vector.tensor_scalar_mul(out=mneg, in0=m, scalar1=-1.0)
                    eP = eP_p.tile([128, R], f16, tag="eP", name="eP")
                    Z = small.tile([128, 1], f32, tag="Z", name="Z")
                    nc.scalar.activation(out=eP, in_=lP[pr], func=AF.Exp,
                                         bias=mneg[:, 0:1], scale=1.0, accum_out=Z)
                    for ce in range(2):
                        nc.sync.dma_start(out=zq[:, 2 * pr + ce:2 * pr + ce + 1],
                                          in_=Z[64 * ce:64 * (ce + 1), 0:1])
                    for rb in range(R // 128):
                        psT2 = psT_p.tile([128, 128], f16, tag="psT", name="psT2")
                        nc.tensor.transpose(
                            psT2, eP[:, 128 * rb:128 * (rb + 1)], idn16)
                        nc.scalar.copy(
                            out=p2T[:, 2 * pr:2 * pr + 2, rb, :],
                            in_=psT2[:].rearrange("p (ce b) -> p ce b", ce=2))
                rzq = small.tile([B, CLOC], f32, tag="rzq", name="rzq")
                nc.vector.reciprocal(out=rzq, in_=zq)

                # --- xe + s matmuls ---
                psS = psacc_p.tile([128, B], f32, tag="psS", name="psS")
                for j in range(CIN):
                    xes = []
                    for c4 in range(CLOC):
                        xe = xe_p.tile([128, R // 128, B], f16, tag=f"xe{c4}",
                                       name=f"xe{c4}")
                        nc.vector.tensor_mul(
                            out=xe,
                            in0=xt2_sb[:, 16 * j:16 * (j + 1), :],
                            in1=p2T[:, c4, :, :])
                        xes.append(xe)
                    for rb in range(R // 128):
                        k = 16 * j + rb
                        for c4 in range(CLOC):
                            nc.tensor.matmul(
                                psS[32 * c4:32 * (c4 + 1), :],
                                w2_sb[:, k, 32 * c4:32 * (c4 + 1)],
                                xes[c4][:, rb, :],
                                start=(k == 0), stop=(k == NK - 1),
                                tile_position=(0, 32 * c4))
                # psS [(c,o), b] -> [b, (c,o)]
                sSt = small.tile([128, B], f32, tag="sSt", name="sSt")
                nc.scalar.copy(out=sSt, in_=psS)
                psSb = psT_p.tile([B, 128], f32, tag="psT", name="psSb")
                nc.tensor.transpose(psSb, sSt, idn)
                sSb = small.tile([B, 128], f32, tag="sSb", name="sSb")
                nc.scalar.copy(out=sSb, in_=psSb)
                out_i, outT = squash(sSb, rz=rzq)

            nc.sync.dma_start(out=out3[:], in_=out_i)

    nc.finalize()
    return nc


def _get_program():
    if "nc" not in _CACHE:
        _CACHE["nc"] = _build_program()
    return _CACHE["nc"]


def make_in_maps(x, route_weights):
    x = np.ascontiguousarray(x, dtype=np.float32)
    W = np.ascontiguousarray(route_weights, dtype=np.float32)
    # [(j,r), b] -> [rj%128, k, b]
    xt2 = x.transpose(2, 1, 0).reshape(RJ, B)
    xt2r = np.ascontiguousarray(
        xt2.reshape(NK, 128, B).transpose(1, 0, 2)).astype(np.float16)
    xnat = x.reshape(B, RJ)                                     # [b, (r,j)]
    x2d = np.ascontiguousarray(
        np.concatenate([xnat, xnat], axis=0)).astype(np.float16)
    ident = np.eye(128, dtype=np.float32)
    in_maps = []
    for core in range(NCORES):
        wc = W[CLOC * core:CLOC * (core + 1)]                   # [4,R,J,O]
        # [(j,r),(c,o)] -> [rj%128, k, (c,o)]
        w2cat = wc.transpose(2, 1, 0, 3).reshape(RJ, CLOC * OUT)
        w2r = np.ascontiguousarray(
            w2cat.reshape(NK, 128, CLOC * OUT).transpose(1, 0, 2)).astype(np.float16)
        # [(c,o), (r,j)]
        wtc = np.ascontiguousarray(
            wc.transpose(0, 3, 1, 2).reshape(CLOC * OUT, RJ)).astype(np.float16)
        m = {"w2r": w2r, "xt2r": xt2r, "wt": wtc, "x2d": x2d, "ident": ident}
        in_maps.append(m)
    return in_maps


def kernel(x, route_weights):
    from concourse.bass_utils import run_bass_kernel_spmd

    in_maps = make_in_maps(x, route_weights)
    nc = _get_program()
    res = run_bass_kernel_spmd(nc, in_maps, core_ids=list(range(NCORES)))
    if os.environ.get("CAPS_RESULT_STASH"):
        _CACHE["last_result"] = res

    out = np.empty((C, B, 1, 1, OUT), dtype=np.float32)
    for core in range(NCORES):
        o = res.results[core]["out3"].reshape(B, CLOC, OUT).transpose(1, 0, 2)
        out[CLOC * core:CLOC * (core + 1), :, 0, 0, :] = o
    return out
